# revision 9
# baseline (speedup 1.0000x reference)
"""LIF spike scan kernel for Trainium2, SPMD over 8 NeuronCores.

Problem: x [B=64, T=8, C=128, H=32, W=32] f32.  Per (b,c,h,w) pixel, scan
over T:  v = tau*u + x_t ; s_t = (v > 1) ; u = v*(v <= 1).  Output spikes
[B, T, C, H, W] f32.

Design: the scan is a pure-DVE sequential chain over two "super-groups"
(SG) of 4 batch rows each, with all element-wise ops fused to FD=4096
(4 rows x 1024 pixels) to amortize DVE per-op overhead:
    g = (v <= 4096) * 0.5     tensor_scalar  i16 -> f16 {0,0.5}   4x mode
    m = v * g                 tensor_tensor  i16 x f16 -> i16     2x_1P
    v' = m + q_t              tensor_tensor  i16 + i16 (in place) 2x_1P
Spikes for steps 0-6 of a group are bit-packed into one byte by the PE
(pack += 2^(t+1) * g_t in PSUM f32, bit t = keep), copied per 512-chunk to
u8 right after each stop-matmul (all hidden under step 7's DVE work), so
the only exposed tail is the raw f16 g_7 plane DMA: step 7 has no mult
(no later step) so its gate is shipped directly instead of packed,
removing 16 serial tail matmuls.  Host ships q = round(x * 2^12) int16
(threshold 4096 = 1.0); i16 writeback rounds to nearest even; numerics
are identical to the two-op-per-step reference quantization (rel err
1.54e-2 < 2e-2 gate, 2202/67M flipped spikes).

Sharding: pure batch-parallel across 8 cores, no collectives.
"""

import numpy as np

B, T, C, HW = 64, 8, 128, 32 * 32
N_CORES = 8
B_LOC = B // N_CORES          # 8 batch rows per core
SCALE = 2.0 ** -12
THI = 4096.0                  # threshold in scaled domain
NSG = 2                       # super-groups per core
SGB = B_LOC // NSG            # batch rows per super-group (4)
FP = SGB * HW                 # fused free dim (4096)
FG = FP // 2                  # pack free dim per group (2048)

_cache = {}


def _build_nc():
    from concourse import bacc, mybir, tile

    op = mybir.AluOpType
    nc = bacc.Bacc(
        "TRN2", target_bir_lowering=False, debug=False, num_devices=N_CORES
    )
    i16, f16, f32 = mybir.dt.int16, mybir.dt.float16, mybir.dt.float32
    u8 = mybir.dt.uint8
    # q pre-shuffled on host to [sg*T + t, c, (g bl hw)]
    x_ext = nc.dram_tensor(
        "x", [NSG * T, C, FP], i16, kind="ExternalInput"
    ).ap()
    # Pack weights: w[:, t*C:(t+1)*C] = 2^(t+1) * I  (f16, exact), t=0..6
    w_ext = nc.dram_tensor("w", [C, (T - 1) * C], f16,
                           kind="ExternalInput").ap()
    # One byte-plane per group: bit t = keep at step t (t=0..6).
    out_ext = nc.dram_tensor(
        "out", [NSG * 2, C, FG], u8, kind="ExternalOutput"
    ).ap()
    # Raw step-7 gates {0, 0.5} f16, one plane per super-group.
    g7_ext = nc.dram_tensor(
        "g7", [NSG, C, FP], f16, kind="ExternalOutput"
    ).ap()

    with tile.TileContext(nc) as tc:
        with tc.tile_pool(name="pool", bufs=2) as pool, tc.tile_pool(
            name="psum", bufs=2, space="PSUM"
        ) as ppool:
            wt = pool.tile([C, (T - 1) * C], f16, tag="w", bufs=1)
            xc = {}
            for sg in range(NSG):
                for t in range(T):
                    xc[t] = pool.tile(
                        [C, FP], i16, tag="x", bufs=5, name=f"x{sg}_{t}"
                    )
                    if sg == 0 and t == 0:
                        # quarter the critical first load, alternating two
                        # queues so per-transfer latencies pipeline; the
                        # weight load rides the scalar queue afterwards
                        # (not needed until the first matmul)
                        for k in range(4):
                            eng = nc.sync if k % 2 == 0 else nc.scalar
                            eng.dma_start(
                                out=xc[t][:, k * 1024 : (k + 1) * 1024],
                                in_=x_ext[0, :, k * 1024 : (k + 1) * 1024],
                            )
                        nc.scalar.dma_start(out=wt, in_=w_ext)
                    else:
                        nc.sync.dma_start(out=xc[t], in_=x_ext[sg * T + t])
                pk = [
                    ppool.tile([C, FG], f32, tag="pk", name=f"pk{sg}_{g}")
                    for g in range(2)
                ]
                pu = [
                    pool.tile([C, FG], u8, tag="pu", bufs=4, name=f"pu{sg}_{g}")
                    for g in range(2)
                ]
                for t in range(T):
                    if t > 0:
                        # v = m + q_t (in place over the x slot)
                        nc.vector.tensor_tensor(
                            out=xc[t], in0=mt, in1=xc[t], op=op.add
                        )
                    gt = pool.tile([C, FP], f16, tag="g", bufs=4,
                                   name=f"g{sg}_{t}")
                    # keep-gate with tau folded in: {0, 0.5} f16 (4x).
                    # First tile: quarters, to start as soon as the first
                    # quarter-load lands.  Last step: quarters, so each g7
                    # slice's DMA is issued while the rest still compute
                    # (hides the ~3us per-transfer DMA latency in the tail).
                    nsplit = 4 if (sg == 0 and t == 0) or t == T - 1 else 1
                    for j in range(0, FP, FP // nsplit):
                        nc.vector.tensor_scalar(
                            out=gt[:, j : j + FP // nsplit],
                            in0=xc[t][:, j : j + FP // nsplit],
                            scalar1=THI, scalar2=0.5,
                            op0=op.is_le, op1=op.mult,
                        )
                        if t == T - 1:
                            eng = nc.sync if (j // 1024) % 2 == 0 else nc.scalar
                            eng.dma_start(
                                out=g7_ext[sg, :, j : j + FP // nsplit],
                                in_=gt[:, j : j + FP // nsplit],
                            )
                    if t < T - 1:
                        mt = pool.tile([C, FP], i16, tag="m", bufs=2,
                                       name=f"m{sg}_{t}")
                        # m = v * g  (reset + tau; i16 x f16, 2x_1P)
                        for j in range(0, FP, FP // nsplit):
                            nc.vector.tensor_tensor(
                                out=mt[:, j : j + FP // nsplit],
                                in0=xc[t][:, j : j + FP // nsplit],
                                in1=gt[:, j : j + FP // nsplit],
                                op=op.mult,
                            )
                        # pack += 2^(t+1) * g  (PE, f32 PSUM, exact); after
                        # the stop-MM (t=6) copy each chunk out immediately —
                        # all of it hides under step 7's DVE ops
                        for g in range(2):
                            for j in range(0, FG, 512):
                                nc.tensor.matmul(
                                    pk[g][:, j : j + 512],
                                    wt[:, t * C : (t + 1) * C],
                                    gt[:, g * FG + j : g * FG + j + 512],
                                    start=(t == 0),
                                    stop=(t == T - 2),
                                )
                                if t == T - 2:
                                    nc.scalar.copy(
                                        out=pu[g][:, j : j + 512],
                                        in_=pk[g][:, j : j + 512],
                                    )
                    # step 7: no mult, no pack — the gate shipped raw above
                for g in range(2):
                    nc.sync.dma_start(out=out_ext[sg * 2 + g], in_=pu[g])
    nc.compile()
    return nc


def _run(x: np.ndarray, trace: bool = False, tmpdir=None):
    from concourse.bass_utils import run_bass_kernel_spmd

    if "nc" not in _cache:
        _cache["nc"] = _build_nc()
    nc = _cache["nc"]
    x = np.asarray(x)
    q = np.clip(np.rint(x * np.float32(1.0 / SCALE)), -32768, 32767).astype(
        np.int16
    )
    # q[b=(sg*4+g*2+bl), t, c, hw] -> [core, (sg t), c, (g bl hw)]
    q7 = q.reshape(N_CORES, NSG, 2, 2, T, C, HW)
    q_shuf = np.ascontiguousarray(q7.transpose(0, 1, 4, 5, 2, 3, 6)).reshape(
        N_CORES, NSG * T, C, FP
    )
    w = np.zeros((C, (T - 1) * C), dtype=np.float16)
    for t in range(T - 1):
        w[np.arange(C), t * C + np.arange(C)] = np.float16(2.0 ** (t + 1))
    in_maps = [{"x": q_shuf[i], "w": w} for i in range(N_CORES)]
    res = run_bass_kernel_spmd(
        nc, in_maps, core_ids=list(range(N_CORES)), trace=trace, tmpdir=tmpdir
    )
    _cache["last_results"] = res
    outs = [res.results[i]["out"] for i in range(N_CORES)]
    g7s = [res.results[i]["g7"] for i in range(N_CORES)]
    # bytes [core, (sg g), c, (bl hw)]; bit t = keep at step t, t=0..6
    by = np.stack(outs, axis=0).reshape(N_CORES, NSG, 2, 1, C, 2, HW)
    by = by.astype(np.uint8)
    tl = np.arange(T - 1, dtype=np.uint8).reshape(1, 1, 1, T - 1, 1, 1, 1)
    keep = (by >> tl) & np.uint8(1)        # [core, sg, g, t, c, bl, hw]
    spk = (1 - keep).astype(np.float32)
    # step-7 spikes from the raw gates: spike = (g7 == 0)
    g7 = np.stack(g7s, axis=0).reshape(N_CORES, NSG, C, 2, 2, HW)
    s7 = (g7 == 0).astype(np.float32).transpose(0, 1, 3, 2, 4, 5)
    s7 = s7.reshape(N_CORES, NSG, 2, 1, C, 2, HW)
    spk = np.concatenate([spk, s7], axis=3)  # [core, sg, g, T, c, bl, hw]
    out = spk.transpose(0, 1, 2, 5, 3, 4, 6).reshape(B, T, C, HW)
    return np.ascontiguousarray(out).reshape(B, T, C, 32, 32)


def kernel(x: np.ndarray) -> np.ndarray:
    return _run(x, trace=False)


# revision 10
# speedup vs baseline: 1.0298x; 1.0298x over previous
"""LIF spike scan kernel for Trainium2, SPMD over 8 NeuronCores.

Problem: x [B=64, T=8, C=128, H=32, W=32] f32.  Per (b,c,h,w) pixel, scan
over T:  v = tau*u + x_t ; s_t = (v > 1) ; u = v*(v <= 1).  Output spikes
[B, T, C, H, W] f32.

Design: all-int16 scaled domain; the scan itself is 3 packed DVE ops per
step, and the output is bit-packed by the (otherwise idle) PE+ACT engines
so HBM write traffic drops 8x.  The recurrence is scale-invariant, so the
host ships q = round(x * 2^12) int16 and the device scans integer membrane
state (threshold 4096 = 1.0).  Per step, with m = tau*u the pre-halved
carry:
    v = m + q_t               DVE tensor_tensor add   i16 x i16 -> 2x_1P
    g = (v <= 4096) * 0.5     DVE tensor_scalar dual  i16 -> fp16 {0,0.5} 4x
    m = v * g                 DVE tensor_tensor mult  i16 x fp16 -> 2x_1P
    pack += 2^(tl+1) * g      PE matmul diag(2^(tl+1)) @ g -> f32 PSUM
After 4 steps the PSUM byte-plane holds sum(2^tl * keep_tl) in [0,15]; ACT
copies it to uint8 and one small DMA ships it.  Host decodes
spike(t=h*4+tl) = 1 - bit tl of byte[h].  Groups run in pairs so two
[C,2048] f32 pack accumulators exactly fill the 8 PSUM banks.
tau=0.5 keeps v dyadic, compares vs 4096 are exact, i16 writeback
saturates and rounds-to-nearest-even (hw-verified): 2202 flipped spikes
of 9.3M vs the f32 reference (rel 1.54e-2 < 2e-2 gate) from input
quantization + halving ties.

Sharding: pure batch-parallel across 8 cores, no collectives.
"""

import numpy as np

B, T, C, HW = 64, 8, 128, 32 * 32
N_CORES = 8
B_LOC = B // N_CORES
SCALE = 2.0 ** -12
THI = 4096.0  # threshold in scaled domain
GB = 2        # batch rows per scan group (F = GB*HW = 2048 free dim)
NG = B_LOC // GB
TH = T // 2   # t-steps per half-chunk

_cache = {}


def _build_nc():
    from concourse import bacc, mybir, tile

    op = mybir.AluOpType
    nc = bacc.Bacc(
        "TRN2", target_bir_lowering=False, debug=False, num_devices=N_CORES
    )
    i16, f16, f32 = mybir.dt.int16, mybir.dt.float16, mybir.dt.float32
    u8 = mybir.dt.uint8
    F = GB * HW
    # q pre-shuffled on host to [g*2+h, c, (tl bl hw)]: contiguous 2D loads.
    x_ext = nc.dram_tensor(
        "x", [NG * 2, C, TH * F], i16, kind="ExternalInput"
    ).ap()
    # Pack weights: w[:, tl*C:(tl+1)*C] = 2^(tl+1) * I  (fp16, exact)
    w_ext = nc.dram_tensor(
        "w", [C, TH * C], f16, kind="ExternalInput"
    ).ap()
    # Output: one byte-plane per (group, half): bit tl = keep at t=h*4+tl.
    out_ext = nc.dram_tensor(
        "out", [NG * 2, C, F], u8, kind="ExternalOutput"
    ).ap()

    with tile.TileContext(nc) as tc:
        with tc.tile_pool(name="pool", bufs=2) as pool, tc.tile_pool(
            name="psum", bufs=2, space="PSUM"
        ) as ppool:
            wt = pool.tile([C, TH * C], f16, tag="w", bufs=1)
            # load pack weights on the Scalar queue so the first x chunk
            # owns the Sync queue from cycle 0
            nc.scalar.dma_start(out=wt, in_=w_ext)
            # Per-group membrane carry m = tau*u, persists across halves.
            mt = [
                pool.tile([C, F], i16, tag=f"m{g}", bufs=1, name=f"m{g}")
                for g in range(NG)
            ]
            xc = {}
            for h in range(2):
                for pair in range(NG // 2):
                    gs = (2 * pair, 2 * pair + 1)
                    for g in gs:
                        xc[g] = pool.tile(
                            [C, TH * F], i16, tag="x", bufs=6, name=f"x{h}_{g}"
                        )
                    # breadth-first quarter loads
                    for tl in range(TH):
                        for g in gs:
                            lo = tl * F
                            nc.sync.dma_start(
                                out=xc[g][:, lo : lo + F],
                                in_=x_ext[g * 2 + h, :, lo : lo + F],
                            )
                    pk = {
                        g: ppool.tile([C, F], f32, tag="pk", name=f"pk{h}_{g}")
                        for g in gs
                    }
                    for tl in range(TH):
                        t = h * TH + tl
                        vs = {g: xc[g][:, tl * F : (tl + 1) * F] for g in gs}
                        gt = {
                            g: pool.tile(
                                [C, F], f16, tag="g", bufs=8, name=f"g{t}_{g}"
                            )
                            for g in gs
                        }
                        if t > 0:
                            for g in gs:
                                # v = m + q_t (in place; i16 2x)
                                nc.vector.tensor_tensor(
                                    out=vs[g], in0=mt[g], in1=vs[g], op=op.add
                                )
                        for g in gs:
                            # keep-gate with tau folded in: {0,0.5} fp16 (4x)
                            nc.vector.tensor_scalar(
                                out=gt[g], in0=vs[g], scalar1=THI, scalar2=0.5,
                                op0=op.is_le, op1=op.mult,
                            )
                        if t < T - 1:
                            for g in gs:
                                # m = v * g  (reset + tau; i16 x fp16 2x)
                                nc.vector.tensor_tensor(
                                    out=mt[g], in0=vs[g], in1=gt[g], op=op.mult
                                )
                        for g in gs:
                            # pack += 2^(tl+1) * g  (PE, f32 PSUM, exact;
                            # moving free dim capped at 512)
                            for j in range(0, F, 512):
                                nc.tensor.matmul(
                                    pk[g][:, j : j + 512],
                                    wt[:, tl * C : (tl + 1) * C],
                                    gt[g][:, j : j + 512],
                                    start=(tl == 0),
                                    stop=(tl == TH - 1),
                                )
                    for g in gs:
                        pu = pool.tile(
                            [C, F], u8, tag="pu", bufs=4, name=f"p{h}{g}"
                        )
                        nc.scalar.copy(out=pu, in_=pk[g])
                        nc.scalar.dma_start(out=out_ext[g * 2 + h], in_=pu)
    nc.compile()
    return nc


def _run(x: np.ndarray, trace: bool = False, tmpdir=None):
    from concourse.bass_utils import run_bass_kernel_spmd

    if "nc" not in _cache:
        _cache["nc"] = _build_nc()
    nc = _cache["nc"]
    x = np.asarray(x)
    q = np.clip(np.rint(x * np.float32(1.0 / SCALE)), -32768, 32767).astype(
        np.int16
    )
    # q[b=(g*GB+bl), t=(h*TH+tl), c, hw] -> [core, g, h, c, tl, bl, hw]
    q6 = q.reshape(N_CORES, NG, GB, 2, TH, C, HW)
    q_shuf = np.ascontiguousarray(q6.transpose(0, 1, 3, 5, 4, 2, 6)).reshape(
        N_CORES, NG * 2, C, TH * GB * HW
    )
    w = np.zeros((C, TH * C), dtype=np.float16)
    for tl in range(TH):
        w[np.arange(C), tl * C + np.arange(C)] = np.float16(2.0 ** (tl + 1))
    in_maps = [{"x": q_shuf[i], "w": w} for i in range(N_CORES)]
    res = run_bass_kernel_spmd(
        nc, in_maps, core_ids=list(range(N_CORES)), trace=trace, tmpdir=tmpdir
    )
    _cache["last_results"] = res
    outs = [res.results[i]["out"] for i in range(N_CORES)]
    # bytes [core, g*2+h, c, (bl hw)]; bit tl = keep at t = h*4+tl
    by = np.stack(outs, axis=0).reshape(N_CORES, NG, 2, 1, C, GB, HW)
    by = by.astype(np.uint8)
    tl_idx = np.arange(TH, dtype=np.uint8).reshape(1, 1, 1, TH, 1, 1, 1)
    keep = (by >> tl_idx) & np.uint8(1)           # [core, g, h, tl, c, bl, hw]
    spk = (1 - keep).astype(np.float32)
    out = spk.transpose(0, 1, 5, 2, 3, 4, 6).reshape(B, T, C, HW)
    return np.ascontiguousarray(out).reshape(B, T, C, 32, 32)


def kernel(x: np.ndarray) -> np.ndarray:
    return _run(x, trace=False)



# revision 11
# speedup vs baseline: 1.0440x; 1.0139x over previous
"""LIF spike scan kernel for Trainium2, SPMD over 8 NeuronCores.

Problem: x [B=64, T=8, C=128, H=32, W=32] f32.  Per (b,c,h,w) pixel, scan
over T:  v = tau*u + x_t ; s_t = (v > 1) ; u = v*(v <= 1).  Output spikes
[B, T, C, H, W] f32.

Design: all-int16 scaled domain (host ships q = round(x * 2^12) i16,
threshold 4096 = 1.0).  Two pairs of 2-row groups run sequentially; the
two groups (A, B) of a pair interleave on the DVE queue so every adjacent
DVE op is independent — consecutive ops pipeline (~70ns overlap each)
instead of serializing on the 8-slice DRAIN.  Per step and group:
    v = m + q_t               tensor_tensor  i16 + i16 (in place)  2x_1P
    g = (v <= 4096) * 0.5     tensor_scalar  i16 -> f16 {0,0.5}    4x
    m = v * g                 tensor_tensor  i16 x f16 -> i16      2x_1P
Steps 0-6 are bit-packed into ONE byte per pixel by the otherwise-idle
PE (pack += 2^(t+1) * g_t in f32 PSUM; bit t = keep), with each 512-wide
chunk copied to u8 right after its stop-matmul at t=6 — fully hidden
under step 7's DVE work.  Step 7 has no mult (no later step), so its
gate plane is shipped raw as f16 instead of packed: this removes all 16
tail matmuls and leaves a ~1MB DMA as the only exposed tail.  The first
input chunk is split across the sync/scalar queues by group so group A's
scan starts as soon as its half lands.  i16 writeback rounds to nearest
even; 2202/67M flipped spikes vs the f32 reference (rel 1.54e-2 < 2e-2).

Sharding: pure batch-parallel across 8 cores, no collectives.
"""

import numpy as np

B, T, C, HW = 64, 8, 128, 32 * 32
N_CORES = 8
B_LOC = B // N_CORES          # 8 batch rows per core
SCALE = 2.0 ** -12
THI = 4096.0                  # threshold in scaled domain
NSG = 2                       # pairs per core
FG = 2 * HW                   # free dim per group (2 rows x 1024 = 2048)
FP = 2 * FG                   # free dim per pair chunk (4096)

_cache = {}


def _build_nc():
    from concourse import bacc, mybir, tile

    op = mybir.AluOpType
    nc = bacc.Bacc(
        "TRN2", target_bir_lowering=False, debug=False, num_devices=N_CORES
    )
    i16, f16, f32 = mybir.dt.int16, mybir.dt.float16, mybir.dt.float32
    u8 = mybir.dt.uint8
    # q pre-shuffled on host to [sg*T + t, c, (g bl hw)]
    x_ext = nc.dram_tensor(
        "x", [NSG * T, C, FP], i16, kind="ExternalInput"
    ).ap()
    # Pack weights: w[:, t*C:(t+1)*C] = 2^(t+1) * I  (f16, exact), t=0..6
    w_ext = nc.dram_tensor("w", [C, (T - 1) * C], f16,
                           kind="ExternalInput").ap()
    # One byte-plane per group: bit t = keep at step t (t=0..6).
    out_ext = nc.dram_tensor(
        "out", [NSG * 2, C, FG], u8, kind="ExternalOutput"
    ).ap()
    # Raw step-7 gates {0, 0.5} f16, one plane per pair.
    g7_ext = nc.dram_tensor(
        "g7", [NSG, C, FP], f16, kind="ExternalOutput"
    ).ap()

    with tile.TileContext(nc) as tc:
        with tc.tile_pool(name="pool", bufs=2) as pool, tc.tile_pool(
            name="psum", bufs=2, space="PSUM"
        ) as ppool:
            wt = pool.tile([C, (T - 1) * C], f16, tag="w", bufs=1)
            xc = {}
            for sg in range(NSG):
                for t in range(T):
                    xc[t] = pool.tile(
                        [C, FP], i16, tag="x", bufs=5, name=f"x{sg}_{t}"
                    )
                    if sg == 0 and t == 0:
                        # split the critical first load by group across two
                        # queues: group A scans as soon as its half lands
                        nc.sync.dma_start(
                            out=xc[t][:, 0:FG], in_=x_ext[0, :, 0:FG]
                        )
                        nc.scalar.dma_start(
                            out=xc[t][:, FG:FP], in_=x_ext[0, :, FG:FP]
                        )
                        nc.scalar.dma_start(out=wt, in_=w_ext)
                    else:
                        nc.sync.dma_start(out=xc[t], in_=x_ext[sg * T + t])
                pk = [
                    ppool.tile([C, FG], f32, tag="pk", name=f"pk{sg}_{g}")
                    for g in range(2)
                ]
                pu = [
                    pool.tile([C, FG], u8, tag="pu", bufs=4, name=f"pu{sg}_{g}")
                    for g in range(2)
                ]
                sl = [slice(0, FG), slice(FG, FP)]   # group A / B columns
                mt = {}
                for t in range(T):
                    if t > 0:
                        # v = m + q_t (in place over the x slot)
                        for g in range(2):
                            nc.vector.tensor_tensor(
                                out=xc[t][:, sl[g]], in0=mt[g],
                                in1=xc[t][:, sl[g]], op=op.add,
                            )
                    gt = [
                        pool.tile([C, FG], f16, tag="g", bufs=8,
                                  name=f"g{sg}_{t}_{g}")
                        for g in range(2)
                    ]
                    # keep-gate with tau folded in: {0, 0.5} f16 (4x)
                    for g in range(2):
                        nc.vector.tensor_scalar(
                            out=gt[g], in0=xc[t][:, sl[g]],
                            scalar1=THI, scalar2=0.5,
                            op0=op.is_le, op1=op.mult,
                        )
                    if t < T - 1:
                        for g in range(2):
                            mt[g] = pool.tile([C, FG], i16, tag="m", bufs=4,
                                              name=f"m{sg}_{t}_{g}")
                            # m = v * g  (reset + tau; i16 x f16, 2x_1P)
                            nc.vector.tensor_tensor(
                                out=mt[g], in0=xc[t][:, sl[g]], in1=gt[g],
                                op=op.mult,
                            )
                        # pack += 2^(t+1) * g  (PE, f32 PSUM, exact); after
                        # the stop-MM (t=6) copy each chunk out immediately —
                        # all of it hides under step 7's DVE work
                        for g in range(2):
                            for j in range(0, FG, 512):
                                nc.tensor.matmul(
                                    pk[g][:, j : j + 512],
                                    wt[:, t * C : (t + 1) * C],
                                    gt[g][:, j : j + 512],
                                    start=(t == 0),
                                    stop=(t == T - 2),
                                )
                                if t == T - 2:
                                    nc.scalar.copy(
                                        out=pu[g][:, j : j + 512],
                                        in_=pk[g][:, j : j + 512],
                                    )
                    else:
                        # step 7: no mult, no pack — ship the gates raw on
                        # both queues in parallel
                        nc.sync.dma_start(
                            out=g7_ext[sg, :, 0:FG], in_=gt[0]
                        )
                        nc.scalar.dma_start(
                            out=g7_ext[sg, :, FG:FP], in_=gt[1]
                        )
                for g in range(2):
                    nc.sync.dma_start(out=out_ext[sg * 2 + g], in_=pu[g])
    nc.compile()
    return nc


def _run(x: np.ndarray, trace: bool = False, tmpdir=None):
    from concourse.bass_utils import run_bass_kernel_spmd

    if "nc" not in _cache:
        _cache["nc"] = _build_nc()
    nc = _cache["nc"]
    x = np.asarray(x)
    q = np.clip(np.rint(x * np.float32(1.0 / SCALE)), -32768, 32767).astype(
        np.int16
    )
    # q[b=(sg*4+g*2+bl), t, c, hw] -> [core, (sg t), c, (g bl hw)]
    q7 = q.reshape(N_CORES, NSG, 2, 2, T, C, HW)
    q_shuf = np.ascontiguousarray(q7.transpose(0, 1, 4, 5, 2, 3, 6)).reshape(
        N_CORES, NSG * T, C, FP
    )
    w = np.zeros((C, (T - 1) * C), dtype=np.float16)
    for t in range(T - 1):
        w[np.arange(C), t * C + np.arange(C)] = np.float16(2.0 ** (t + 1))
    in_maps = [{"x": q_shuf[i], "w": w} for i in range(N_CORES)]
    res = run_bass_kernel_spmd(
        nc, in_maps, core_ids=list(range(N_CORES)), trace=trace, tmpdir=tmpdir
    )
    _cache["last_results"] = res
    outs = [res.results[i]["out"] for i in range(N_CORES)]
    g7s = [res.results[i]["g7"] for i in range(N_CORES)]
    # bytes [core, (sg g), c, (bl hw)]; bit t = keep at step t, t=0..6
    by = np.stack(outs, axis=0).reshape(N_CORES, NSG, 2, 1, C, 2, HW)
    by = by.astype(np.uint8)
    tl = np.arange(T - 1, dtype=np.uint8).reshape(1, 1, 1, T - 1, 1, 1, 1)
    keep = (by >> tl) & np.uint8(1)        # [core, sg, g, t, c, bl, hw]
    spk = (1 - keep).astype(np.float32)
    # step-7 spikes from the raw gates: spike = (g7 == 0)
    g7 = np.stack(g7s, axis=0).reshape(N_CORES, NSG, C, 2, 2, HW)
    s7 = (g7 == 0).astype(np.float32).transpose(0, 1, 3, 2, 4, 5)
    s7 = s7.reshape(N_CORES, NSG, 2, 1, C, 2, HW)
    spk = np.concatenate([spk, s7], axis=3)  # [core, sg, g, T, c, bl, hw]
    out = spk.transpose(0, 1, 2, 5, 3, 4, 6).reshape(B, T, C, HW)
    return np.ascontiguousarray(out).reshape(B, T, C, 32, 32)


def kernel(x: np.ndarray) -> np.ndarray:
    return _run(x, trace=False)
